# revision 10
# baseline (speedup 1.0000x reference)
"""MoIE (mixture of implicit experts) Trainium2 kernel.

Math (per reference):
    alpha = softmax(x @ gate_W + gate_b)                    # (B, K)
    h = x
    for l in 0..3:  h = relu(sum_k alpha_k * (h @ W[l,k] + b[l,k]))
    out = sum_k alpha_k * (h @ out_W[k] + out_b[k])

Strategy:
  - Data-parallel: shard B=32768 tokens over 8 cores (4096 each); replicate
    the small weights. No collectives.
  - Feature-major on device: activations live as hT [D(part), T(free)] so
    chained matmuls need no activation transposes (weights are the stationary
    operand in natural [i, o] layout).
  - alpha folded into the *moving* operand: rhs_k = hT * bcast(alphaT[k]).
    PSUM then accumulates over experts AND contraction chunks in one group;
    the per-expert bias enters as a tiny alphaT-contraction matmul
    (sum_k alpha[t,k] b[k,o] == b.T-as-lhsT @ alphaT).
  - fp16 on the matmul/scaling path (1 cycle/row on PE, 2x mode on DVE;
    values are O(1) so fp16 range is safe), fp32 PSUM accumulation,
    gate/softmax in fp32 (+f32r for the tiny reduction matmuls).
"""

import sys

if "/opt/trn_rl_repo" not in sys.path:
    sys.path.insert(0, "/opt/trn_rl_repo")

import numpy as np

import concourse.bass as bass
import concourse.tile as tile
import concourse.mybir as mybir
from concourse import bacc
from concourse.bass_utils import run_bass_kernel_spmd

N_CORES = 8
B, D, K, L = 32768, 256, 8, 4
NL = L + 1                  # 4 hidden blocks + output block
BS = B // N_CORES           # 4096 tokens per core
T = 2048                    # tokens per on-chip tile
NT = BS // T                # tiles per core
SEG = 512                   # f32 PSUM bank = 512 elements
NSEG = T // SEG
F16 = mybir.dt.float16
F32 = mybir.dt.float32
F32R = mybir.dt.float32r
AF = mybir.ActivationFunctionType


def _build_kernel():
    nc = bacc.Bacc(
        "TRN2",
        target_bir_lowering=False,
        debug=False,
        enable_asserts=False,
        num_devices=N_CORES,
    )
    xT = nc.dram_tensor("xT", [D, BS], F16, kind="ExternalInput").ap()
    w = nc.dram_tensor("w", [NL, K, D, D], F16, kind="ExternalInput").ap()
    bb = nc.dram_tensor("bb", [NL, K, D], F16, kind="ExternalInput").ap()
    gw = nc.dram_tensor("gw", [D, K], F16, kind="ExternalInput").ap()
    gb = nc.dram_tensor("gb", [1, K], F16, kind="ExternalInput").ap()
    oh = nc.dram_tensor("oh", [K, K * 128], F16, kind="ExternalInput").ap()
    outT = nc.dram_tensor("outT", [D, BS], F32, kind="ExternalOutput").ap()

    with tile.TileContext(nc) as tc:
        _body(nc, tc, xT, w, bb, gw, gb, oh, outT)
    nc.compile()
    return nc


def _body(nc, tc, xT, w, bb, gw, gb, oh, outT):
    with (
        tc.tile_pool(name="cpool", bufs=1) as cpool,
        tc.tile_pool(name="hpool", bufs=6) as hpool,
        tc.tile_pool(name="rpool", bufs=6) as rpool,
        tc.tile_pool(name="apool", bufs=1) as apool,
        tc.tile_pool(name="spool", bufs=2) as spool,
        tc.tile_pool(name="opool", bufs=2) as opool,
        tc.tile_pool(name="ppool", bufs=2, space=bass.MemorySpace.PSUM) as ppool,
    ):
        # ---- resident constants / weights ----
        wt = cpool.tile([128, NL * K * 2 * D], F16, name="wt")
        for l in range(NL):
            for k in range(K):
                for i2 in range(2):
                    off = ((l * K + k) * 2 + i2) * D
                    nc.sync.dma_start(
                        wt[:, off : off + D], w[l, k, i2 * 128 : (i2 + 1) * 128, :]
                    )
        bt = cpool.tile([K, NL * D], F16, name="bt")
        for l in range(NL):
            nc.sync.dma_start(bt[:, l * D : (l + 1) * D], bb[l])
        gwt = cpool.tile([128, 2 * K], F16, name="gwt")
        for i2 in range(2):
            nc.sync.dma_start(
                gwt[:, i2 * K : (i2 + 1) * K], gw[i2 * 128 : (i2 + 1) * 128, :]
            )
        gbt = cpool.tile([1, K], F16, name="gbt")
        nc.sync.dma_start(gbt[:], gb[:])
        oht = cpool.tile([K, K * 128], F16, name="oht")
        nc.sync.dma_start(oht[:], oh[:])
        ones_row = cpool.tile([1, T], F16, name="ones_row")
        nc.vector.memset(ones_row[:], 1.0)
        ones8x8 = cpool.tile([K, K], F16, name="ones8x8")
        nc.vector.memset(ones8x8[:], 1.0)

        def wslice(l, k, i2, o2):
            base = ((l * K + k) * 2 + i2) * D + o2 * 128
            return wt[:, base : base + 128]

        for ti in range(NT):
            t0 = ti * T
            # ---- load x tile (feature-major) ----
            h = []
            for i2 in range(2):
                ht = hpool.tile([128, T], F16, tag="h", name=f"x_{ti}_{i2}")
                nc.sync.dma_start(ht[:], xT[i2 * 128 : (i2 + 1) * 128, t0 : t0 + T])
                h.append(ht)

            # ---- gate logits glT[k, t] = gate_W.T @ x + gate_b ----
            glT = ppool.tile([K, T], F32, tag="z", name=f"glT_{ti}")
            for s in range(NSEG):
                sl = slice(s * SEG, (s + 1) * SEG)
                nc.tensor.matmul(
                    glT[:, sl], gwt[:, 0:K], h[0][:, sl], start=True, stop=False
                )
                nc.tensor.matmul(
                    glT[:, sl], gwt[:, K : 2 * K], h[1][:, sl], start=False, stop=False
                )
                nc.tensor.matmul(
                    glT[:, sl], gbt[:], ones_row[:, sl], start=False, stop=True
                )

            # ---- softmax over the 8 partitions (no max-subtract needed;
            #      logits are ~N(0,1) so exp() is safe in fp32) ----
            eT = spool.tile([K, T], F16, tag="eT", name=f"eT_{ti}")
            nc.scalar.activation(eT[:], glT[:], AF.Exp)
            # sum over experts, broadcast back to all 8 partitions in one go:
            # all-ones [8,8] lhsT -> every output row is sum_k e_k
            sT8 = ppool.tile([K, T], F32, tag="z", name=f"sT8_{ti}")
            for s in range(NSEG):
                sl = slice(s * SEG, (s + 1) * SEG)
                nc.tensor.matmul(
                    sT8[:, sl], ones8x8[:], eT[:, sl], start=True, stop=True
                )
            r8 = spool.tile([K, T], F16, tag="rT", name=f"r8_{ti}")
            with nc.allow_low_precision("fp16 softmax normalizer is within tolerance"):
                nc.vector.reciprocal(r8[:], sT8[:])
            alphaT = spool.tile([K, T], F16, tag="alphaT", name=f"alphaT_{ti}")
            nc.vector.tensor_mul(alphaT[:], eT[:], r8[:])

            # ---- broadcast alphaT rows across all 128 partitions:
            #      abc[:, k*T:(k+1)*T] = onehot_k.T @ alphaT ----
            abc = apool.tile([128, K * T], F16, tag="abc", name=f"abc_{ti}")
            for k in range(K):
                pk = ppool.tile([128, T], F32, tag="z", name=f"abc_ps_{ti}_{k}")
                for s in range(NSEG):
                    sl = slice(s * SEG, (s + 1) * SEG)
                    nc.tensor.matmul(
                        pk[:, sl],
                        oht[:, k * 128 : (k + 1) * 128],
                        alphaT[:, sl],
                        start=True,
                        stop=True,
                    )
                nc.scalar.activation(abc[:, k * T : (k + 1) * T], pk[:], AF.Copy)

            # ---- the 5 blocks ----
            for l in range(NL):
                rhs = {}
                for k in range(K):
                    for i2 in range(2):
                        rt = rpool.tile([128, T], F16, tag="rhs", name=f"rhs_{ti}_{l}_{k}_{i2}")
                        nc.vector.tensor_mul(
                            rt[:], h[i2][:], abc[:, k * T : (k + 1) * T]
                        )
                        rhs[k, i2] = rt
                z = []
                for o2 in range(2):
                    zt = ppool.tile([128, T], F32, tag="z", name=f"z_{ti}_{l}_{o2}")
                    z.append(zt)
                # bias first: z[o2] = b[l].T-as-lhsT @ alphaT  (PE-warm buffer
                # while the DVE produces the first scaled rhs)
                for o2 in range(2):
                    for s in range(NSEG):
                        sl = slice(s * SEG, (s + 1) * SEG)
                        nc.tensor.matmul(
                            z[o2][:, sl],
                            bt[:, l * D + o2 * 128 : l * D + (o2 + 1) * 128],
                            alphaT[:, sl],
                            start=True,
                            stop=False,
                        )
                for k in range(K):
                    for i2 in range(2):
                        last = (k == K - 1) and (i2 == 1)
                        for o2 in range(2):
                            for s in range(NSEG):
                                sl = slice(s * SEG, (s + 1) * SEG)
                                nc.tensor.matmul(
                                    z[o2][:, sl],
                                    wslice(l, k, i2, o2),
                                    rhs[k, i2][:, sl],
                                    start=False,
                                    stop=last,
                                )
                if l < NL - 1:
                    newh = []
                    for o2 in range(2):
                        nh = hpool.tile([128, T], F16, tag="h", name=f"h_{ti}_{l}_{o2}")
                        nc.scalar.activation(nh[:], z[o2][:], AF.Relu)
                        newh.append(nh)
                    h = newh
                else:
                    for o2 in range(2):
                        ot = opool.tile([128, T], F32, tag="o", name=f"out_{ti}_{o2}")
                        nc.scalar.activation(ot[:], z[o2][:], AF.Copy)
                        nc.sync.dma_start(
                            outT[o2 * 128 : (o2 + 1) * 128, t0 : t0 + T], ot[:]
                        )


_NC_CACHE = None


def _get_nc():
    global _NC_CACHE
    if _NC_CACHE is None:
        _NC_CACHE = _build_kernel()
    return _NC_CACHE


class _Runner:
    """Persistent sharded PJRT executable for the bass kernel (compile once,
    run many). Mirrors bass2jax.run_bass_via_pjrt's multi-core branch minus
    buffer donation (the kernel writes every output element)."""

    def __init__(self):
        import jax
        from jax.sharding import Mesh, PartitionSpec, NamedSharding
        from jax.experimental.shard_map import shard_map
        from concourse import bass2jax, mybir as _mybir

        self.jax = jax
        nc = _get_nc()
        bass2jax.install_neuronx_cc_hook()
        part_name = nc.partition_id_tensor.name if nc.partition_id_tensor else None
        in_names, out_names, out_avals, zero_outs = [], [], [], []
        for alloc in nc.m.functions[0].allocations:
            if not isinstance(alloc, _mybir.MemoryLocationSet):
                continue
            name = alloc.memorylocations[0].name
            if alloc.kind == "ExternalInput":
                if name != part_name:
                    in_names.append(name)
            elif alloc.kind == "ExternalOutput":
                out_names.append(name)
                shape = tuple(alloc.tensor_shape)
                dtype = _mybir.dt.np(alloc.dtype)
                out_avals.append(jax.core.ShapedArray(shape, dtype))
                zero_outs.append(np.zeros(shape, dtype))
        self.in_names, self.out_names, self.out_avals = in_names, out_names, out_avals

        bind_names = in_names + out_names + ([part_name] if part_name else [])

        def _body(*args):
            operands = list(args)
            if part_name is not None:
                operands.append(bass2jax.partition_id_tensor())
            outs = bass2jax._bass_exec_p.bind(
                *operands,
                out_avals=tuple(out_avals),
                in_names=tuple(bind_names),
                out_names=tuple(out_names),
                lowering_input_output_aliases=(),
                sim_require_finite=True,
                sim_require_nnan=True,
                nc=nc,
            )
            return tuple(outs)

        devices = jax.devices()[:N_CORES]
        self.mesh = Mesh(np.asarray(devices), ("core",))
        self.spec = PartitionSpec("core")
        self.sharding = NamedSharding(self.mesh, self.spec)
        n_args = len(in_names) + len(out_names)
        self.fn = jax.jit(
            shard_map(
                _body,
                mesh=self.mesh,
                in_specs=(self.spec,) * n_args,
                out_specs=(self.spec,) * len(out_names),
                check_rep=False,
            ),
            keep_unused=True,
        )
        self.zero_outs = [
            jax.device_put(
                np.zeros((N_CORES * z.shape[0], *z.shape[1:]), z.dtype), self.sharding
            )
            for z in zero_outs
        ]

    def device_inputs(self, in_maps):
        concat = [
            np.concatenate([np.asarray(m[name]) for m in in_maps], axis=0)
            for name in self.in_names
        ]
        return [self.jax.device_put(a, self.sharding) for a in concat]

    def run(self, dev_in):
        outs = self.fn(*dev_in, *self.zero_outs)
        return outs

    def to_maps(self, outs):
        res = []
        for c in range(N_CORES):
            res.append(
                {
                    name: np.asarray(outs[i]).reshape(
                        N_CORES, *self.out_avals[i].shape
                    )[c]
                    for i, name in enumerate(self.out_names)
                }
            )
        return res


_RUNNER = None


def _get_runner():
    global _RUNNER
    if _RUNNER is None:
        _RUNNER = _Runner()
    return _RUNNER


def _make_in_maps(x, gate_W, gate_b, block_W, block_b, out_W, out_b):
    x = np.asarray(x, dtype=np.float32)
    xT = np.ascontiguousarray(x.T).astype(np.float16)            # [D, B]
    w_all = np.concatenate(
        [np.asarray(block_W, np.float32), np.asarray(out_W, np.float32)[None]], axis=0
    ).astype(np.float16)                                          # [NL, K, D, D]
    b_all = np.concatenate(
        [np.asarray(block_b, np.float32), np.asarray(out_b, np.float32)[None]], axis=0
    ).astype(np.float16)                                          # [NL, K, D]
    gw = np.asarray(gate_W, np.float32).astype(np.float16)        # [D, K]
    gb = np.asarray(gate_b, np.float32).astype(np.float16).reshape(1, K)
    oh = np.zeros((K, K * 128), np.float16)
    for k in range(K):
        oh[k, k * 128 : (k + 1) * 128] = 1.0
    in_maps = []
    for c in range(N_CORES):
        in_maps.append(
            {
                "xT": np.ascontiguousarray(xT[:, c * BS : (c + 1) * BS]),
                "w": w_all,
                "bb": b_all,
                "gw": gw,
                "gb": gb,
                "oh": oh,
            }
        )
    return in_maps


def _assemble(results):
    parts = [np.asarray(results[c]["outT"], np.float32).T for c in range(N_CORES)]
    return np.ascontiguousarray(np.concatenate(parts, axis=0))


def kernel(x, gate_W, gate_b, block_W, block_b, out_W, out_b):
    runner = _get_runner()
    in_maps = _make_in_maps(x, gate_W, gate_b, block_W, block_b, out_W, out_b)
    dev_in = runner.device_inputs(in_maps)
    outs = runner.run(dev_in)
    return _assemble(runner.to_maps(outs))


def bench(x, gate_W, gate_b, block_W, block_b, out_W, out_b, iters=20):
    """Returns (output, per_iteration_ns) — steady-state pipelined device time."""
    import time as _time

    runner = _get_runner()
    in_maps = _make_in_maps(x, gate_W, gate_b, block_W, block_b, out_W, out_b)
    dev_in = runner.device_inputs(in_maps)
    outs = runner.run(dev_in)  # warm-up + compile
    for o in outs:
        o.block_until_ready()
    t0 = _time.perf_counter()
    all_outs = [runner.run(dev_in) for _ in range(iters)]
    for outs_i in all_outs:
        for o in outs_i:
            o.block_until_ready()
    t1 = _time.perf_counter()
    per_iter_ns = (t1 - t0) / iters * 1e9
    return _assemble(runner.to_maps(all_outs[-1])), per_iter_ns


# revision 55
# speedup vs baseline: 224.2506x; 224.2506x over previous
"""MoIE (mixture of implicit experts) Trainium2 kernel.

Math (per reference):
    alpha = softmax(x @ gate_W + gate_b)                    # (B, K)
    h = x
    for l in 0..3:  h = relu(sum_k alpha_k * (h @ W[l,k] + b[l,k]))
    out = sum_k alpha_k * (h @ out_W[k] + out_b[k])

Strategy:
  - Data-parallel: shard B=32768 tokens over 8 cores (4096 each); replicate
    the small weights. No collectives.
  - Feature-major on device: activations live as hT [D(part), T(free)] so
    chained matmuls need no activation transposes (weights are the stationary
    operand in natural [i, o] layout).
  - alpha folded into the *moving* operand: rhs_k = hT * bcast(alphaT[k]).
    PSUM then accumulates over experts AND contraction chunks in one group;
    the per-expert bias enters as a tiny alphaT-contraction matmul
    (sum_k alpha[t,k] b[k,o] == b.T-as-lhsT @ alphaT).
  - fp16 on the matmul/scaling path (1 cycle/row on PE, 2x mode on DVE;
    values are O(1) so fp16 range is safe), fp32 PSUM accumulation,
    gate/softmax in fp32 (+f32r for the tiny reduction matmuls).
"""

import sys

if "/opt/trn_rl_repo" not in sys.path:
    sys.path.insert(0, "/opt/trn_rl_repo")

import numpy as np

import concourse.bass as bass
import concourse.tile as tile
import concourse.mybir as mybir
from concourse import bacc
from concourse.bass_utils import run_bass_kernel_spmd

N_CORES = 8
B, D, K, L = 32768, 256, 8, 4
NL = L + 1                  # 4 hidden blocks + output block
BS = B // N_CORES           # 4096 tokens per core
T = 2048                    # tokens per on-chip tile
NT = BS // T                # tiles per core
SEG = 512                   # f32 PSUM bank = 512 elements
NSEG = T // SEG
F16 = mybir.dt.float16
F32 = mybir.dt.float32
F32R = mybir.dt.float32r
AF = mybir.ActivationFunctionType
_ABL = None  # ablation switch for perf bisection ('nodve', 'nostagea')
_APOOL_BUFS = 2
_RPOOL_BUFS = 5
_DVE_EVAC = False   # evacuate PSUM on DVE instead of ACT
_BCAST_ENGINE = "sync"  # which engine queue issues the broadcast DMAs


def _build_kernel(reps=1):
    nc = bacc.Bacc(
        "TRN2",
        target_bir_lowering=False,
        debug=False,
        enable_asserts=False,
        num_devices=N_CORES,
    )
    xT = nc.dram_tensor("xT", [D, BS], F16, kind="ExternalInput").ap()
    w = nc.dram_tensor("w", [NL, K, D, D], F16, kind="ExternalInput").ap()
    bb = nc.dram_tensor("bb", [NL, K, D], F16, kind="ExternalInput").ap()
    gw = nc.dram_tensor("gw", [D, K], F16, kind="ExternalInput").ap()
    gb = nc.dram_tensor("gb", [1, K], F16, kind="ExternalInput").ap()
    outT = nc.dram_tensor("outT", [D, BS], F32, kind="ExternalOutput").ap()

    with tile.TileContext(nc) as tc:
        _body(nc, tc, xT, w, bb, gw, gb, outT, reps)
    nc.compile()
    return nc


def _body(nc, tc, xT, w, bb, gw, gb, outT, reps=1):
    with (
        tc.tile_pool(name="cpool", bufs=1) as cpool,
        tc.tile_pool(name="hpool", bufs=6) as hpool,
        tc.tile_pool(name="rpool", bufs=_RPOOL_BUFS) as rpool,
        tc.tile_pool(name="apool", bufs=_APOOL_BUFS) as apool,
        tc.tile_pool(name="spool", bufs=2) as spool,
        tc.tile_pool(name="opool", bufs=2) as opool,
        tc.tile_pool(name="dpool", bufs=2, space=bass.MemorySpace.DRAM) as dpool,
        tc.tile_pool(name="ppool", bufs=2, space=bass.MemorySpace.PSUM) as ppool,
    ):
        # ---- small constants first (the HWDGE queue is FIFO: keep the
        # gate/bias/x transfers ahead of the 5MB weight stream) ----
        gwt = cpool.tile([128, 2 * K], F16, name="gwt")
        for i2 in range(2):
            nc.sync.dma_start(
                gwt[:, i2 * K : (i2 + 1) * K], gw[i2 * 128 : (i2 + 1) * 128, :]
            )
        gbt = cpool.tile([1, K], F16, name="gbt")
        nc.sync.dma_start(gbt[:], gb[:])
        bt = cpool.tile([K, NL * D], F16, name="bt")
        ones_row = cpool.tile([1, T], F16, name="ones_row")
        nc.vector.memset(ones_row[:], 1.0)
        ones8x8 = cpool.tile([K, K], F16, name="ones8x8")
        nc.vector.memset(ones8x8[:], 1.0)
        wt = cpool.tile([128, NL * K * 2 * D], F16, name="wt")

        def load_weights():
            for l in range(NL):
                nc.sync.dma_start(bt[:, l * D : (l + 1) * D], bb[l])
            for l in range(NL):
                for k in range(K):
                    for i2 in range(2):
                        off = ((l * K + k) * 2 + i2) * D
                        nc.sync.dma_start(
                            wt[:, off : off + D],
                            w[l, k, i2 * 128 : (i2 + 1) * 128, :],
                        )

        def wslice(l, k, i2, o2):
            base = ((l * K + k) * 2 + i2) * D + o2 * 128
            return wt[:, base : base + 128]

        if reps > 1:
            # steady-state benchmarking variant: weights resident across reps
            load_weights()
            ctx = tc.For_i(0, reps, 1)
            ctx.__enter__()

        for ti in range(NT):
            t0 = ti * T
            # ---- load x tile (feature-major), seg-chunked so the gate
            # matmuls can start on the first 512 tokens ----
            h = []
            for i2 in range(2):
                ht = hpool.tile([128, T], F16, tag="h", name=f"x_{ti}_{i2}")
                for s in range(NSEG):
                    sl = slice(s * SEG, (s + 1) * SEG)
                    nc.sync.dma_start(
                        ht[:, sl], xT[i2 * 128 : (i2 + 1) * 128, t0 + s * SEG : t0 + (s + 1) * SEG]
                    )
                h.append(ht)

            if _ABL is not None and "nostagea" in _ABL:
                alphaT = spool.tile([K, T], F16, tag="alphaT", name=f"alphaTs_{ti}")
                nc.vector.memset(alphaT[:], 0.125)
                abc = apool.tile([128, K * T], F16, tag="abc", name=f"abcs_{ti}")
                nc.vector.memset(abc[:], 0.125)
            else:
                # ---- gate logits glT[k, t] = gate_W.T @ x + gate_b ----
                # (PSUM slots are [128, T/2]-sized; gate runs per half)
                eT = spool.tile([K, T], F16, tag="eT", name=f"eT_{ti}")
                sT8s = []
                for hf in range(2):
                    glT = ppool.tile([K, T // 2], F32, tag="z", name=f"glT_{ti}_{hf}")
                    for s in range(NSEG // 2):
                        sl = slice(s * SEG, (s + 1) * SEG)
                        gsl = slice(hf * (T // 2) + s * SEG, hf * (T // 2) + (s + 1) * SEG)
                        nc.tensor.matmul(
                            glT[:, sl], gwt[:, 0:K], h[0][:, gsl], start=True, stop=False
                        )
                        nc.tensor.matmul(
                            glT[:, sl], gwt[:, K : 2 * K], h[1][:, gsl], start=False, stop=False
                        )
                        nc.tensor.matmul(
                            glT[:, sl], gbt[:], ones_row[:, sl], start=False, stop=True
                        )
                    # softmax over the 8 partitions (no max-subtract needed;
                    # logits are ~N(0,1) so exp() is safe in fp32)
                    hsl = slice(hf * (T // 2), (hf + 1) * (T // 2))
                    nc.scalar.activation(eT[:, hsl], glT[:], AF.Exp)
                    # sum over experts, broadcast back to all 8 partitions in
                    # one go: all-ones [8,8] lhsT -> every row is sum_k e_k
                    sT8 = ppool.tile([K, T // 2], F32, tag="z", name=f"sT8_{ti}_{hf}")
                    for s in range(NSEG // 2):
                        sl = slice(s * SEG, (s + 1) * SEG)
                        esl = slice(hf * (T // 2) + s * SEG, hf * (T // 2) + (s + 1) * SEG)
                        nc.tensor.matmul(
                            sT8[:, sl], ones8x8[:], eT[:, esl], start=True, stop=True
                        )
                    sT8s.append(sT8)
                r8 = spool.tile([K, T], F16, tag="rT", name=f"r8_{ti}")
                with nc.allow_low_precision("fp16 softmax normalizer"):
                    for hf in range(2):
                        hsl = slice(hf * (T // 2), (hf + 1) * (T // 2))
                        nc.vector.reciprocal(r8[:, hsl], sT8s[hf][:])
                alphaT = spool.tile([K, T], F16, tag="alphaT", name=f"alphaT_{ti}")
                nc.vector.tensor_mul(alphaT[:], eT[:], r8[:])

                if _ABL is not None and "sa_noabc" in _ABL:
                    abc = apool.tile([128, K * T], F16, tag="abc", name=f"abcn_{ti}")
                    nc.vector.memset(abc[:], 0.125)
                elif _ABL is not None and "sa_nobcast" in _ABL:
                    ast = spool.tile([1, K * T], F16, tag="ast", name=f"ast_{ti}", bufs=1)
                    abc = apool.tile([128, K * T], F16, tag="abc", name=f"abcb_{ti}")
                    nc.vector.memset(abc[:], 0.125)
                    for k in range(K):
                        nc.sync.dma_start(
                            ast[:, k * T : (k + 1) * T], alphaT[k : k + 1, :]
                        )
                elif _ABL is not None and "sa_nodma" in _ABL:
                    ast = spool.tile([1, K * T], F16, tag="ast", name=f"ast_{ti}", bufs=1)
                    nc.vector.memset(ast[:], 0.125)
                    abc = apool.tile([128, K * T], F16, tag="abc", name=f"abcd_{ti}")
                    for k in range(K):
                        nc.gpsimd.partition_broadcast(
                            abc[:, k * T : (k + 1) * T], ast[:, k * T : (k + 1) * T]
                        )
                else:
                    # broadcast alphaT rows across partitions with step-0
                    # DMA reads: bounce alphaT through DRAM (SBUF-source
                    # broadcast APs are unsupported), then 8 parallel
                    # DRAM->SBUF broadcast DMAs — no compute engine involved
                    adram = dpool.tile([K, T], F16, tag="adram", name=f"adram_{ti}")
                    nc.sync.dma_start(adram[:], alphaT[:])
                    abc = apool.tile([128, K * T], F16, tag="abc", name=f"abc_{ti}")
                    beng = getattr(nc, _BCAST_ENGINE)
                    for k in range(K):
                        beng.dma_start(
                            abc[:, k * T : (k + 1) * T],
                            adram[k : k + 1, :].broadcast_to([128, T]),
                        )

            if ti == 0 and reps == 1:
                load_weights()

            # ---- the 5 blocks ----
            if _ABL is not None and "nolayers" in _ABL:
                for o2 in range(2):
                    ot = opool.tile([128, T], F32, tag="o", name=f"outn_{ti}_{o2}")
                    nc.vector.tensor_copy(ot[:, 0:T], abc[:, o2 * T : (o2 + 1) * T])
                    nc.sync.dma_start(
                        outT[o2 * 128 : (o2 + 1) * 128, t0 : t0 + T], ot[:]
                    )
                continue
            for l in range(NL):
                rhs = {}
                for k in range(K):
                    for i2 in range(2):
                        if _ABL is not None and "nodve" in _ABL:
                            rhs[k, i2] = h[i2]
                            continue
                        rt = rpool.tile([128, T], F16, tag="rhs", name=f"rhs_{ti}_{l}_{k}_{i2}")
                        nc.vector.tensor_mul(
                            rt[:], h[i2][:], abc[:, k * T : (k + 1) * T]
                        )
                        rhs[k, i2] = rt
                HT = T // 2
                z = {}
                for o2 in range(2):
                    zt = ppool.tile([128, T], F32, tag="z", name=f"z_{ti}_{l}_{o2}")
                    for hf in range(2):
                        z[o2, hf] = zt[:, hf * HT : (hf + 1) * HT]

                def bias_mm(o2):
                    for s in range(NSEG):
                        lsl = slice((s % 2) * SEG, (s % 2 + 1) * SEG)
                        gsl = slice(s * SEG, (s + 1) * SEG)
                        nc.tensor.matmul(
                            z[o2, s // 2][:, lsl],
                            bt[:, l * D + o2 * 128 : l * D + (o2 + 1) * 128],
                            alphaT[:, gsl],
                            start=True,
                            stop=False,
                        )

                def expert_mm(k, i2, o2):
                    last = (k == K - 1) and (i2 == 1)
                    for s in range(NSEG):
                        lsl = slice((s % 2) * SEG, (s % 2 + 1) * SEG)
                        gsl = slice(s * SEG, (s + 1) * SEG)
                        nc.tensor.matmul(
                            z[o2, s // 2][:, lsl],
                            wslice(l, k, i2, o2),
                            rhs[k, i2][:, gsl],
                            start=False,
                            stop=last,
                        )

                # bias(o0) first (only needs the earliest-freed PSUM slots),
                # then the first expert group, then bias(o1) — by which time
                # the o1 slots have been evacuated. Keeps the PE fed across
                # the layer boundary.
                bias_mm(0)
                expert_mm(0, 0, 0)
                bias_mm(1)
                expert_mm(0, 0, 1)
                for k in range(K - 1):
                    for i2 in range(2):
                        if k == 0 and i2 == 0:
                            continue
                        for o2 in range(2):
                            expert_mm(k, i2, o2)
                # final expert sweeps region-by-region (o2, seg) so each PSUM
                # region finishes accumulating early and its evacuation
                # overlaps the rest of the k=7 matmuls instead of serializing
                # at the layer boundary
                for o2 in range(2):
                    for s in range(NSEG):
                        lsl = slice((s % 2) * SEG, (s % 2 + 1) * SEG)
                        gsl = slice(s * SEG, (s + 1) * SEG)
                        for i2 in range(2):
                            nc.tensor.matmul(
                                z[o2, s // 2][:, lsl],
                                wslice(l, K - 1, i2, o2),
                                rhs[K - 1, i2][:, gsl],
                                start=False,
                                stop=(i2 == 1),
                            )

                if _ABL is not None and "noevac" in _ABL:
                    continue
                if l < NL - 1:
                    newh = []
                    for o2 in range(2):
                        nh = hpool.tile([128, T], F16, tag="h", name=f"h_{ti}_{l}_{o2}")
                        for hf in range(2):
                            hsl = slice(hf * HT, (hf + 1) * HT)
                            if _DVE_EVAC:
                                nc.vector.tensor_relu(nh[:, hsl], z[o2, hf][:])
                            else:
                                nc.scalar.activation(nh[:, hsl], z[o2, hf][:], AF.Relu)
                        newh.append(nh)
                    h = newh
                else:
                    for o2 in range(2):
                        ot = opool.tile([128, T], F32, tag="o", name=f"out_{ti}_{o2}")
                        for hf in range(2):
                            hsl = slice(hf * HT, (hf + 1) * HT)
                            if _DVE_EVAC:
                                nc.vector.tensor_copy(ot[:, hsl], z[o2, hf][:])
                            else:
                                nc.scalar.activation(ot[:, hsl], z[o2, hf][:], AF.Copy)
                        nc.sync.dma_start(
                            outT[o2 * 128 : (o2 + 1) * 128, t0 : t0 + T], ot[:]
                        )

        if reps > 1:
            ctx.__exit__(None, None, None)


_NC_CACHE = None


def _get_nc():
    global _NC_CACHE
    if _NC_CACHE is None:
        _NC_CACHE = _build_kernel()
    return _NC_CACHE


class _Runner:
    """Persistent sharded PJRT executable for the bass kernel (compile once,
    run many). Mirrors bass2jax.run_bass_via_pjrt's multi-core branch minus
    buffer donation (the kernel writes every output element)."""

    def __init__(self, nc=None):
        import jax
        from jax.sharding import Mesh, PartitionSpec, NamedSharding
        from jax.experimental.shard_map import shard_map
        from concourse import bass2jax, mybir as _mybir

        self.jax = jax
        if nc is None:
            nc = _get_nc()
        bass2jax.install_neuronx_cc_hook()
        part_name = nc.partition_id_tensor.name if nc.partition_id_tensor else None
        in_names, out_names, out_avals, zero_outs = [], [], [], []
        for alloc in nc.m.functions[0].allocations:
            if not isinstance(alloc, _mybir.MemoryLocationSet):
                continue
            name = alloc.memorylocations[0].name
            if alloc.kind == "ExternalInput":
                if name != part_name:
                    in_names.append(name)
            elif alloc.kind == "ExternalOutput":
                out_names.append(name)
                shape = tuple(alloc.tensor_shape)
                dtype = _mybir.dt.np(alloc.dtype)
                out_avals.append(jax.core.ShapedArray(shape, dtype))
                zero_outs.append(np.zeros(shape, dtype))
        self.in_names, self.out_names, self.out_avals = in_names, out_names, out_avals

        bind_names = in_names + out_names + ([part_name] if part_name else [])

        def _body(*args):
            operands = list(args)
            if part_name is not None:
                operands.append(bass2jax.partition_id_tensor())
            outs = bass2jax._bass_exec_p.bind(
                *operands,
                out_avals=tuple(out_avals),
                in_names=tuple(bind_names),
                out_names=tuple(out_names),
                lowering_input_output_aliases=(),
                sim_require_finite=True,
                sim_require_nnan=True,
                nc=nc,
            )
            return tuple(outs)

        devices = jax.devices()[:N_CORES]
        self.mesh = Mesh(np.asarray(devices), ("core",))
        self.spec = PartitionSpec("core")
        self.sharding = NamedSharding(self.mesh, self.spec)
        n_args = len(in_names) + len(out_names)
        self.fn = jax.jit(
            shard_map(
                _body,
                mesh=self.mesh,
                in_specs=(self.spec,) * n_args,
                out_specs=(self.spec,) * len(out_names),
                check_rep=False,
            ),
            keep_unused=True,
        )
        self.zero_outs = [
            jax.device_put(
                np.zeros((N_CORES * z.shape[0], *z.shape[1:]), z.dtype), self.sharding
            )
            for z in zero_outs
        ]

    def device_inputs(self, in_maps):
        concat = [
            np.concatenate([np.asarray(m[name]) for m in in_maps], axis=0)
            for name in self.in_names
        ]
        return [self.jax.device_put(a, self.sharding) for a in concat]

    def run(self, dev_in):
        outs = self.fn(*dev_in, *self.zero_outs)
        return outs

    def to_maps(self, outs):
        res = []
        for c in range(N_CORES):
            res.append(
                {
                    name: np.asarray(outs[i]).reshape(
                        N_CORES, *self.out_avals[i].shape
                    )[c]
                    for i, name in enumerate(self.out_names)
                }
            )
        return res


_RUNNER = None


def _get_runner():
    global _RUNNER
    if _RUNNER is None:
        _RUNNER = _Runner()
    return _RUNNER


def _make_in_maps(x, gate_W, gate_b, block_W, block_b, out_W, out_b):
    x = np.asarray(x, dtype=np.float32)
    xT = np.ascontiguousarray(x.T).astype(np.float16)            # [D, B]
    w_all = np.concatenate(
        [np.asarray(block_W, np.float32), np.asarray(out_W, np.float32)[None]], axis=0
    ).astype(np.float16)                                          # [NL, K, D, D]
    b_all = np.concatenate(
        [np.asarray(block_b, np.float32), np.asarray(out_b, np.float32)[None]], axis=0
    ).astype(np.float16)                                          # [NL, K, D]
    gw = np.asarray(gate_W, np.float32).astype(np.float16)        # [D, K]
    gb = np.asarray(gate_b, np.float32).astype(np.float16).reshape(1, K)
    in_maps = []
    for c in range(N_CORES):
        in_maps.append(
            {
                "xT": np.ascontiguousarray(xT[:, c * BS : (c + 1) * BS]),
                "w": w_all,
                "bb": b_all,
                "gw": gw,
                "gb": gb,
            }
        )
    return in_maps


def _assemble(results):
    parts = [np.asarray(results[c]["outT"], np.float32).T for c in range(N_CORES)]
    return np.ascontiguousarray(np.concatenate(parts, axis=0))


def kernel(x, gate_W, gate_b, block_W, block_b, out_W, out_b):
    runner = _get_runner()
    in_maps = _make_in_maps(x, gate_W, gate_b, block_W, block_b, out_W, out_b)
    dev_in = runner.device_inputs(in_maps)
    outs = runner.run(dev_in)
    return _assemble(runner.to_maps(outs))


def bench(x, gate_W, gate_b, block_W, block_b, out_W, out_b, iters=20):
    """Returns (output, per_iteration_ns) — steady-state pipelined device time."""
    import time as _time

    runner = _get_runner()
    in_maps = _make_in_maps(x, gate_W, gate_b, block_W, block_b, out_W, out_b)
    dev_in = runner.device_inputs(in_maps)
    outs = runner.run(dev_in)  # warm-up + compile
    for o in outs:
        o.block_until_ready()
    t0 = _time.perf_counter()
    all_outs = [runner.run(dev_in) for _ in range(iters)]
    for outs_i in all_outs:
        for o in outs_i:
            o.block_until_ready()
    t1 = _time.perf_counter()
    per_iter_ns = (t1 - t0) / iters * 1e9
    return _assemble(runner.to_maps(all_outs[-1])), per_iter_ns
